# revision 8
# baseline (speedup 1.0000x reference)
"""Grouped SwiGLU MoE FFN (8 experts) on 8 Trainium2 NeuronCores.

Expert-parallel: core e owns expert e's weights and its contiguous slice of
tokens (inputs arrive pre-sorted by expert).  Per core we compute
    g = silu(x_e @ w1_e.T); u = x_e @ w3_e.T; y_e = (g*u) @ w2_e.T

Matmuls run on the PE array as fp8(e4m3) DoubleRow pairs at 0.5 cycles/row,
2x the bf16/fp32r rate.  To stay inside the 2e-2 error budget each operand
is split into hi+lo e4m3 parts (a "Karatsuba" split): for y = a@b we compute
a_hi@b_hi + a_lo@b_hi + a_hi@b_lo and drop the lo@lo term, giving ~bf16
accuracy at 0.75x the bf16 PE cost.  Each DoubleRow instruction packs two
independent 128-deep products: hi@hi terms pair adjacent contraction strips
(k, k+1); the two cross terms for one strip share one instruction via
stationary slot order (lo,hi) against moving slot order (hi,lo).

Weights are pre-scaled by 2^8 on the host so their hi parts sit in e4m3's
normal range; the scale is folded back in on the Act engine (silu input
scale and the final psum->bf16 copy).  All hi/lo splitting and layout
packing for x/w1/w3/w2 happens on the host in numpy; the device sees fp8
operands laid out partition-major and streams:
  phase A: per h-strip j, per 512-token tile: g,u psums -> silu (ACT),
           h = g*u (DVE) -> h_hi, h_lo fp8 tiles (DVE)
  phase B: per d-strip i: y psum over 12 padded h-strips -> bf16 out.
"""

import sys

sys.path.insert(0, "/opt/trn_rl_repo")

import numpy as np
import ml_dtypes

import concourse.bass as bass
import concourse.mybir as mybir
import concourse.tile as tile
from concourse import bacc
from concourse.bass import ts
from concourse.bass_utils import run_bass_kernel_spmd

F32 = mybir.dt.float32
BF16 = mybir.dt.bfloat16
F8 = mybir.dt.float8e4
NP_F8 = ml_dtypes.float8_e4m3fn
DR = mybir.MatmulPerfMode.DoubleRow
MULT = mybir.AluOpType.mult
SUBTRACT = mybir.AluOpType.subtract

E, H, D, T = 8, 1408, 2048, 16384
TE = T // E            # tokens per expert (uniform fast path)
KD = D // 128          # 16 contraction strips over d
JH = H // 128          # 11 h strips
JH2 = JH + 1           # padded to even for DoubleRow hi@hi pairing in mm2
ID = D // 128          # 16 output d strips
NT = TE // 512         # 4 token tiles of 512
WS = 256.0             # weight pre-scale (2^8) for e4m3 range health
WARMUP_N = 110         # dummy PE matmuls to cover startup DMA + pstate ramp


def _build_program():
    nc = bacc.Bacc("TRN2", target_bir_lowering=False, debug=False, num_devices=E)

    # [p, k, sl, t]: sl 0=hi, 1=lo of x[t, 128k+p]
    x_d = nc.dram_tensor("xq", [128, KD, 2, TE], F8, kind="ExternalInput").ap()
    # [p, j, s, k, l, m]: s 0=w1,1=w3; l 0=lo,1=hi of (WS*w)[128j+m, 128k+p]
    w13_d = nc.dram_tensor("w13q", [128, JH, 2, KD, 2, 128], F8,
                           kind="ExternalInput").ap()
    # [p, i, kk, l, m]: l 0=lo,1=hi of (WS*w2)[128i+m, 128kk+p]; kk=11 zero pad
    w2_d = nc.dram_tensor("w2q", [128, ID, JH2, 2, 128], F8,
                          kind="ExternalInput").ap()
    # [i, p, t] = y[t, 128i+p]
    y_d = nc.dram_tensor("y", [ID, 128, TE], BF16, kind="ExternalOutput").ap()

    with tile.TileContext(nc) as tc:
        with (
            tc.tile_pool(name="xp", bufs=1) as xp,
            tc.tile_pool(name="wp", bufs=3) as wp,
            tc.tile_pool(name="w2p", bufs=5) as w2p,
            tc.tile_pool(name="hp", bufs=1) as hp,
            tc.tile_pool(name="sp", bufs=2) as sp,
            tc.tile_pool(name="fp", bufs=2) as fp,
            tc.tile_pool(name="yp", bufs=2) as yp,
            tc.tile_pool(name="psA", bufs=2, space="PSUM") as psA,
            tc.tile_pool(name="psB", bufs=3, space="PSUM") as psB,
            tc.tile_pool(name="psW", bufs=1, space="PSUM") as psW,
        ):
            # h strips in fp8 hi/lo; strip JH (=11) is the zero pad for mm2
            # hi@hi pairing (its w2 slot is also zero, but memset keeps any
            # stale NaN encodings out of the pair).
            h = hp.tile([128, JH2, 2, TE], F8, tag="h")
            nc.vector.memset(h[:, JH, 0, :], 0.0)

            # PE warmup: dummy DoubleRow matmuls on a zeroed fp8 tile keep the
            # PE busy (and its pstate ramping toward full clock) while the
            # first weight/activation DMAs stream in.
            wu = sp.tile([128, 2, 128], F8, tag="wu", bufs=1)
            nc.vector.memset(wu[:], 0.0)
            pw = psW.tile([128, 128], F32, tag="pw")
            for _ in range(WARMUP_N):
                nc.tensor.matmul(pw[:], wu[:], wu[:], start=True, stop=True,
                                 perf_mode=DR)

            # startup DMAs, finest-needed-first so the first real matmuls can
            # begin as soon as possible: w1 strip 0 (g weights), x hi parts of
            # token tile 0, then x lo, then w3 strip 0 (u weights).
            w13_cur = wp.tile([128, 2, KD, 2, 128], F8, tag="w13", name="w13t")
            xt = xp.tile([128, KD, 2, TE], F8, tag="xt")
            nc.sync.dma_start(w13_cur[:, 0], w13_d[:, 0, 0])
            nc.sync.dma_start(xt[:, :, 0, ts(0, 512)], x_d[:, :, 0, ts(0, 512)])
            nc.sync.dma_start(xt[:, :, 1, ts(0, 512)], x_d[:, :, 1, ts(0, 512)])
            nc.sync.dma_start(w13_cur[:, 1], w13_d[:, 0, 1])
            w13_nxt = wp.tile([128, 2, KD, 2, 128], F8, tag="w13", name="w13t")
            nc.sync.dma_start(xt[:, :, :, ts(1, 512)], x_d[:, :, :, ts(1, 512)])
            nc.sync.dma_start(w13_nxt[:], w13_d[:, 1])
            for tt in range(2, NT):
                nc.sync.dma_start(
                    xt[:, :, :, ts(tt, 512)], x_d[:, :, :, ts(tt, 512)]
                )
            # w2 prefetch (behind x/w13 in queue order; needed only in phase B)
            w2_tiles = []
            for i in range(4):
                w2t = w2p.tile([128, JH2, 2, 128], F8, tag="w2", name="w2t")
                nc.sync.dma_start(w2t[:], w2_d[:, i])
                w2_tiles.append(w2t)

            # ---- phase A: g/u matmuls + h build ----
            for j in range(JH):
                w13 = w13_cur
                w13_cur = w13_nxt
                if j + 2 < JH:
                    w13_nxt = wp.tile(
                        [128, 2, KD, 2, 128], F8, tag="w13", name="w13t"
                    )
                    nc.sync.dma_start(w13_nxt[:], w13_d[:, j + 2])
                for tt in range(NT):
                    tsl = ts(tt, 512)
                    pg = psA.tile([128, 512], F32, tag="pg")
                    pu = psA.tile([128, 512], F32, tag="pu")
                    for s, ps_ in ((0, pg), (1, pu)):
                        # hi@hi over strip pairs (k, k+1)
                        for k in range(0, KD, 2):
                            nc.tensor.matmul(
                                ps_[:], w13[:, s, k:k + 2, 1, :],
                                xt[:, k:k + 2, 0, tsl],
                                start=(k == 0), stop=False, perf_mode=DR,
                            )
                        # cross terms: stationary (lo,hi) x moving (hi,lo)
                        for k in range(KD):
                            nc.tensor.matmul(
                                ps_[:], w13[:, s, k, :, :], xt[:, k, :, tsl],
                                start=False, stop=(k == KD - 1), perf_mode=DR,
                            )
                    sg = sp.tile([128, 512], F32, tag="sg")
                    nc.scalar.activation(
                        sg[:], pg[:], mybir.ActivationFunctionType.Silu,
                        scale=1.0 / WS,
                    )
                    hf = fp.tile([128, 512], F32, tag="hf")
                    # hf = (pu * 1/WS) * sg = u * g
                    nc.vector.scalar_tensor_tensor(
                        hf[:], pu[:], 1.0 / WS, sg[:], op0=MULT, op1=MULT
                    )
                    nc.vector.tensor_copy(h[:, j, 0, tsl], hf[:])
                    nc.vector.tensor_sub(h[:, j, 1, tsl], hf[:], h[:, j, 0, tsl])

            # ---- phase B: y matmuls + store ----
            for i in range(ID):
                if i + 4 < ID:
                    w2t = w2p.tile([128, JH2, 2, 128], F8, tag="w2", name="w2t")
                    nc.sync.dma_start(w2t[:], w2_d[:, i + 4])
                    w2_tiles.append(w2t)
                w2 = w2_tiles[i]
                y_sb = yp.tile([128, TE], BF16, tag="ysb")
                for tt in range(NT):
                    tsl = ts(tt, 512)
                    py = psB.tile([128, 512], F32, tag="py")
                    for kk in range(0, JH2, 2):
                        nc.tensor.matmul(
                            py[:], w2[:, kk:kk + 2, 1, :],
                            h[:, kk:kk + 2, 0, tsl],
                            start=(kk == 0), stop=False, perf_mode=DR,
                        )
                    for kk in range(JH):
                        nc.tensor.matmul(
                            py[:], w2[:, kk, :, :], h[:, kk, :, tsl],
                            start=False, stop=(kk == JH - 1), perf_mode=DR,
                        )
                    nc.scalar.activation(
                        y_sb[:, tsl], py[:],
                        mybir.ActivationFunctionType.Copy, scale=1.0 / WS,
                    )
                    if i == ID - 1:
                        # last strip: store per token tile so only the final
                        # 512-token slice trails the last matmul
                        nc.sync.dma_start(y_d[i, :, tsl], y_sb[:, tsl])
                if i < ID - 1:
                    nc.sync.dma_start(y_d[i], y_sb[:])

    nc.compile()
    return nc


_NC = None


def _get_nc():
    global _NC
    if _NC is None:
        _NC = _build_program()
    return _NC


def _hilo(a):
    hi = a.astype(NP_F8)
    lo = (a - hi.astype(np.float32)).astype(NP_F8)
    return hi, lo


def _prep_core_inputs(x_e, w1_e, w3_e, w2_e):
    # xq[p, k, sl, t] with sl=(hi,lo) of x_e[t, 128k+p]
    xr = np.ascontiguousarray(x_e.T).reshape(KD, 128, TE)   # [k, p, t]
    x_hi, x_lo = _hilo(xr)
    xq = np.stack([x_hi, x_lo], axis=2)                      # [k, p, sl, t]
    xq = np.ascontiguousarray(xq.transpose(1, 0, 2, 3))      # [p, k, sl, t]

    # w13q[p, j, s, k, l, m] with l=(lo,hi) of WS*w[128j+m, 128k+p]
    def pack_w13(w):
        r = (w * WS).reshape(JH, 128, KD, 128)               # [j, m, k, p]
        hi, lo = _hilo(r)
        t = np.stack([lo, hi], axis=3)                       # [j, m, k, l, p]
        return t.transpose(4, 0, 2, 3, 1)                    # [p, j, k, l, m]

    w13q = np.stack([pack_w13(w1_e), pack_w13(w3_e)], axis=2)  # [p,j,s,k,l,m]
    w13q = np.ascontiguousarray(w13q)

    # w2q[p, i, kk, l, m] with l=(lo,hi) of WS*w2[128i+m, 128kk+p]; pad kk=11
    r2 = (w2_e * WS).reshape(ID, 128, JH, 128)               # [i, m, kk, p]
    hi2, lo2 = _hilo(r2)
    t2 = np.stack([lo2, hi2], axis=3)                        # [i, m, kk, l, p]
    t2 = t2.transpose(4, 0, 2, 3, 1)                         # [p, i, kk, l, m]
    w2q = np.zeros((128, ID, JH2, 2, 128), dtype=NP_F8)
    w2q[:, :, :JH] = t2
    return {"xq": xq, "w13q": w13q, "w2q": np.ascontiguousarray(w2q)}


def _reference_fallback(w1, w2, w3, x, counts):
    # Exact numpy mirror of the jax reference (incl. scatter-drop / gather-clamp)
    e, h, d = w1.shape
    t = x.shape[0]
    cap = 2 * (t // e)
    counts = counts.astype(np.int64)
    offsets = np.concatenate([[0], np.cumsum(counts)[:-1]])
    eid = np.repeat(np.arange(e), counts)[:t]
    pos = np.arange(t) - offsets[eid]
    buf = np.zeros((e, cap, d), np.float32)
    ok = pos < cap
    buf[eid[ok], pos[ok]] = x[ok]
    out = np.empty((e, cap, d), np.float32)
    for ee in range(e):
        a = buf[ee] @ w1[ee].T
        g = a / (1.0 + np.exp(-a))
        u = buf[ee] @ w3[ee].T
        out[ee] = (g * u) @ w2[ee].T
    pos_c = np.minimum(pos, cap - 1)
    return out[eid, pos_c]


def kernel(w1, w2, w3, x, num_tokens_per_expert):
    w1 = np.asarray(w1, dtype=np.float32)
    w2 = np.asarray(w2, dtype=np.float32)
    w3 = np.asarray(w3, dtype=np.float32)
    x = np.asarray(x, dtype=np.float32)
    counts = np.asarray(num_tokens_per_expert).astype(np.int32)

    if not (x.shape == (T, D) and w1.shape == (E, H, D)
            and np.all(counts == TE)):
        return _reference_fallback(w1, w2, w3, x, counts)

    nc = _get_nc()
    in_maps = []
    for e in range(E):
        in_maps.append(
            _prep_core_inputs(x[e * TE:(e + 1) * TE], w1[e], w3[e], w2[e])
        )
    res = run_bass_kernel_spmd(nc, in_maps, list(range(E)))

    out = np.empty((T, D), dtype=np.float32)
    for e in range(E):
        y = res.results[e]["y"]  # [ID, 128, TE] bf16
        out[e * TE:(e + 1) * TE] = (
            y.astype(np.float32).transpose(2, 0, 1).reshape(TE, D)
        )
    return out


# revision 10
# speedup vs baseline: 1.0336x; 1.0336x over previous
"""Grouped SwiGLU MoE FFN (8 experts) on 8 Trainium2 NeuronCores.

Expert-parallel: core e owns expert e's weights and its contiguous slice of
tokens (inputs arrive pre-sorted by expert).  Per core we compute
    g = silu(x_e @ w1_e.T); u = x_e @ w3_e.T; y_e = (g*u) @ w2_e.T

Matmuls run on the PE array as fp8(e4m3) DoubleRow pairs at 0.5 cycles/row,
2x the bf16/fp32r rate.  To stay inside the 2e-2 error budget each operand
is split into hi+lo e4m3 parts (a "Karatsuba" split): for y = a@b we compute
a_hi@b_hi + a_lo@b_hi + a_hi@b_lo and drop the lo@lo term, giving ~bf16
accuracy at 0.75x the bf16 PE cost.  Each DoubleRow instruction packs two
independent 128-deep products: hi@hi terms pair adjacent contraction strips
(k, k+1); the two cross terms for one strip share one instruction via
stationary slot order (lo,hi) against moving slot order (hi,lo).

Weights are pre-scaled by 2^8 on the host so their hi parts sit in e4m3's
normal range; the scale is folded back in on the Act engine (silu input
scale and the final psum->bf16 copy).  All hi/lo splitting and layout
packing for x/w1/w3/w2 happens on the host in numpy; the device sees fp8
operands laid out partition-major and streams:
  phase A: per h-strip j, per 512-token tile: g,u psums -> silu (ACT),
           h = g*u (DVE) -> h_hi, h_lo fp8 tiles (DVE)
  phase B: per d-strip i: y psum over 12 padded h-strips -> bf16 out.
"""

import sys

sys.path.insert(0, "/opt/trn_rl_repo")

import numpy as np
import ml_dtypes

import concourse.bass as bass
import concourse.mybir as mybir
import concourse.tile as tile
from concourse import bacc
from concourse.bass import ts
from concourse.bass_utils import run_bass_kernel_spmd

F32 = mybir.dt.float32
BF16 = mybir.dt.bfloat16
F8 = mybir.dt.float8e4
NP_F8 = ml_dtypes.float8_e4m3fn
DR = mybir.MatmulPerfMode.DoubleRow
MULT = mybir.AluOpType.mult
SUBTRACT = mybir.AluOpType.subtract

E, H, D, T = 8, 1408, 2048, 16384
TE = T // E            # tokens per expert (uniform fast path)
KD = D // 128          # 16 contraction strips over d
JH = H // 128          # 11 h strips
JH2 = JH + 1           # padded to even for DoubleRow hi@hi pairing in mm2
ID = D // 128          # 16 output d strips
NT = TE // 512         # 4 token tiles of 512
WS = 256.0             # weight pre-scale (2^8) for e4m3 range health
WARMUP_N = 110         # dummy PE matmuls to cover startup DMA + pstate ramp


def _build_program():
    nc = bacc.Bacc("TRN2", target_bir_lowering=False, debug=False, num_devices=E)

    # [p, k, sl, t]: sl 0=hi, 1=lo of x[t, 128k+p]
    x_d = nc.dram_tensor("xq", [128, KD, 2, TE], F8, kind="ExternalInput").ap()
    # [p, j, s, k, l, m]: s 0=w1,1=w3; l 0=lo,1=hi of (WS*w)[128j+m, 128k+p]
    w13_d = nc.dram_tensor("w13q", [128, JH, 2, KD, 2, 128], F8,
                           kind="ExternalInput").ap()
    # [p, i, kk, l, m]: l 0=lo,1=hi of (WS*w2)[128i+m, 128kk+p]; kk=11 zero pad
    w2_d = nc.dram_tensor("w2q", [128, ID, JH2, 2, 128], F8,
                          kind="ExternalInput").ap()
    # [i, p, t] = y[t, 128i+p]
    y_d = nc.dram_tensor("y", [ID, 128, TE], BF16, kind="ExternalOutput").ap()

    with tile.TileContext(nc) as tc:
        with (
            tc.tile_pool(name="xp", bufs=1) as xp,
            tc.tile_pool(name="wp", bufs=5) as wp,
            tc.tile_pool(name="w2p", bufs=5) as w2p,
            tc.tile_pool(name="hp", bufs=1) as hp,
            tc.tile_pool(name="sp", bufs=2) as sp,
            tc.tile_pool(name="fp", bufs=2) as fp,
            tc.tile_pool(name="yp", bufs=2) as yp,
            tc.tile_pool(name="psA", bufs=2, space="PSUM") as psA,
            tc.tile_pool(name="psB", bufs=3, space="PSUM") as psB,
            tc.tile_pool(name="psW", bufs=1, space="PSUM") as psW,
        ):
            # h strips in fp8 hi/lo; strip JH (=11) is the zero pad for mm2
            # hi@hi pairing (its w2 slot is also zero, but memset keeps any
            # stale NaN encodings out of the pair).
            h = hp.tile([128, JH2, 2, TE], F8, tag="h")
            nc.vector.memset(h[:, JH, 0, :], 0.0)

            # PE warmup: dummy DoubleRow matmuls on a zeroed fp8 tile keep the
            # PE busy (and its pstate ramping toward full clock) while the
            # first weight/activation DMAs stream in.
            wu = sp.tile([128, 2, 128], F8, tag="wu", bufs=1)
            nc.vector.memset(wu[:], 0.0)
            pw = psW.tile([128, 128], F32, tag="pw")
            for _ in range(WARMUP_N):
                nc.tensor.matmul(pw[:], wu[:], wu[:], start=True, stop=True,
                                 perf_mode=DR)

            # startup DMAs, finest-needed-first so the first real matmuls can
            # begin as soon as possible: w1 strip 0 (g weights), x hi parts of
            # token tile 0, then w3 strip 0 (u weights), x lo, next strips.
            xt = xp.tile([128, KD, 2, TE], F8, tag="xt")
            w13_q = []

            def w13_fetch(j, split=False):
                t = wp.tile([128, 2, KD, 2, 128], F8, tag="w13", name="w13t")
                if split:
                    nc.sync.dma_start(t[:, 0], w13_d[:, j, 0])
                    nc.sync.dma_start(
                        xt[:, :, 0, ts(0, 512)], x_d[:, :, 0, ts(0, 512)]
                    )
                    nc.sync.dma_start(t[:, 1], w13_d[:, j, 1])
                    nc.sync.dma_start(
                        xt[:, :, 1, ts(0, 512)], x_d[:, :, 1, ts(0, 512)]
                    )
                else:
                    nc.sync.dma_start(t[:], w13_d[:, j])
                w13_q.append(t)

            w13_fetch(0, split=True)
            for j in (1, 2, 3):
                w13_fetch(j)
            # zero pad strip for mm2 (Pool engine; off the DMA path)
            nc.vector.memset(h[:, JH, 0, :], 0.0)

            def emit_block(w13, j, tt, hh_first):
                tsl = ts(tt, 512)
                pg = psA.tile([128, 512], F32, tag="pg")
                pu = psA.tile([128, 512], F32, tag="pu")
                order = ("hh", "cross") if hh_first else ("mix",)
                for phase in order:
                    for s, ps_ in ((0, pg), (1, pu)):
                        if phase in ("hh", "mix"):
                            # hi@hi over strip pairs (k, k+1)
                            for k in range(0, KD, 2):
                                nc.tensor.matmul(
                                    ps_[:], w13[:, s, k:k + 2, 1, :],
                                    xt[:, k:k + 2, 0, tsl],
                                    start=(k == 0), stop=False, perf_mode=DR,
                                )
                        if phase in ("cross", "mix"):
                            # cross terms: stationary (lo,hi) x moving (hi,lo)
                            for k in range(KD):
                                nc.tensor.matmul(
                                    ps_[:], w13[:, s, k, :, :],
                                    xt[:, k, :, tsl],
                                    start=False, stop=(k == KD - 1),
                                    perf_mode=DR,
                                )
                sg = sp.tile([128, 512], F32, tag="sg")
                nc.scalar.activation(
                    sg[:], pg[:], mybir.ActivationFunctionType.Silu,
                    scale=1.0 / WS,
                )
                hf = fp.tile([128, 512], F32, tag="hf")
                # hf = (pu * 1/WS) * sg = u * g
                nc.vector.scalar_tensor_tensor(
                    hf[:], pu[:], 1.0 / WS, sg[:], op0=MULT, op1=MULT
                )
                nc.vector.tensor_copy(h[:, j, 0, tsl], hf[:])
                nc.vector.tensor_sub(h[:, j, 1, tsl], hf[:], h[:, j, 0, tsl])

            # ---- phase A sweep 1: token tile 0 across all strips ----
            # x arrives while the w13 stream feeds the PE; later token tiles
            # stream in between weight strips.
            for j in range(JH):
                if j + 4 < JH:
                    w13_fetch(j + 4)
                emit_block(w13_q[j], j, 0, hh_first=True)
                if j == 0:
                    nc.sync.dma_start(
                        xt[:, :, :, ts(1, 512)], x_d[:, :, :, ts(1, 512)]
                    )
                elif j == 3:
                    nc.sync.dma_start(
                        xt[:, :, :, ts(2, 512)], x_d[:, :, :, ts(2, 512)]
                    )
                elif j == 6:
                    nc.sync.dma_start(
                        xt[:, :, :, ts(3, 512)], x_d[:, :, :, ts(3, 512)]
                    )

            # ---- phase A sweep 2: token tiles 1..3, w13 re-streamed ----
            w13_q.clear()
            w13_fetch(0)
            w13_fetch(1)
            # w2 prefetch (needed only in phase B)
            w2_tiles = []
            for i in range(4):
                w2t = w2p.tile([128, JH2, 2, 128], F8, tag="w2", name="w2t")
                nc.sync.dma_start(w2t[:], w2_d[:, i])
                w2_tiles.append(w2t)
            for j in range(JH):
                if j + 2 < JH:
                    w13_fetch(j + 2)
                for tt in range(1, NT):
                    emit_block(w13_q[j], j, tt, hh_first=False)

            # ---- phase B: y matmuls + store ----
            for i in range(ID):
                if i + 4 < ID:
                    w2t = w2p.tile([128, JH2, 2, 128], F8, tag="w2", name="w2t")
                    nc.sync.dma_start(w2t[:], w2_d[:, i + 4])
                    w2_tiles.append(w2t)
                w2 = w2_tiles[i]
                y_sb = yp.tile([128, TE], BF16, tag="ysb")
                for tt in range(NT):
                    tsl = ts(tt, 512)
                    py = psB.tile([128, 512], F32, tag="py")
                    for kk in range(0, JH2, 2):
                        nc.tensor.matmul(
                            py[:], w2[:, kk:kk + 2, 1, :],
                            h[:, kk:kk + 2, 0, tsl],
                            start=(kk == 0), stop=False, perf_mode=DR,
                        )
                    for kk in range(JH):
                        nc.tensor.matmul(
                            py[:], w2[:, kk, :, :], h[:, kk, :, tsl],
                            start=False, stop=(kk == JH - 1), perf_mode=DR,
                        )
                    nc.scalar.activation(
                        y_sb[:, tsl], py[:],
                        mybir.ActivationFunctionType.Copy, scale=1.0 / WS,
                    )
                    if i == ID - 1:
                        # last strip: store per token tile so only the final
                        # 512-token slice trails the last matmul
                        nc.sync.dma_start(y_d[i, :, tsl], y_sb[:, tsl])
                if i < ID - 1:
                    nc.sync.dma_start(y_d[i], y_sb[:])

    nc.compile()
    return nc


_NC = None


def _get_nc():
    global _NC
    if _NC is None:
        _NC = _build_program()
    return _NC


def _hilo(a):
    hi = a.astype(NP_F8)
    lo = (a - hi.astype(np.float32)).astype(NP_F8)
    return hi, lo


def _prep_core_inputs(x_e, w1_e, w3_e, w2_e):
    # xq[p, k, sl, t] with sl=(hi,lo) of x_e[t, 128k+p]
    xr = np.ascontiguousarray(x_e.T).reshape(KD, 128, TE)   # [k, p, t]
    x_hi, x_lo = _hilo(xr)
    xq = np.stack([x_hi, x_lo], axis=2)                      # [k, p, sl, t]
    xq = np.ascontiguousarray(xq.transpose(1, 0, 2, 3))      # [p, k, sl, t]

    # w13q[p, j, s, k, l, m] with l=(lo,hi) of WS*w[128j+m, 128k+p]
    def pack_w13(w):
        r = (w * WS).reshape(JH, 128, KD, 128)               # [j, m, k, p]
        hi, lo = _hilo(r)
        t = np.stack([lo, hi], axis=3)                       # [j, m, k, l, p]
        return t.transpose(4, 0, 2, 3, 1)                    # [p, j, k, l, m]

    w13q = np.stack([pack_w13(w1_e), pack_w13(w3_e)], axis=2)  # [p,j,s,k,l,m]
    w13q = np.ascontiguousarray(w13q)

    # w2q[p, i, kk, l, m] with l=(lo,hi) of WS*w2[128i+m, 128kk+p]; pad kk=11
    r2 = (w2_e * WS).reshape(ID, 128, JH, 128)               # [i, m, kk, p]
    hi2, lo2 = _hilo(r2)
    t2 = np.stack([lo2, hi2], axis=3)                        # [i, m, kk, l, p]
    t2 = t2.transpose(4, 0, 2, 3, 1)                         # [p, i, kk, l, m]
    w2q = np.zeros((128, ID, JH2, 2, 128), dtype=NP_F8)
    w2q[:, :, :JH] = t2
    return {"xq": xq, "w13q": w13q, "w2q": np.ascontiguousarray(w2q)}


def _reference_fallback(w1, w2, w3, x, counts):
    # Exact numpy mirror of the jax reference (incl. scatter-drop / gather-clamp)
    e, h, d = w1.shape
    t = x.shape[0]
    cap = 2 * (t // e)
    counts = counts.astype(np.int64)
    offsets = np.concatenate([[0], np.cumsum(counts)[:-1]])
    eid = np.repeat(np.arange(e), counts)[:t]
    pos = np.arange(t) - offsets[eid]
    buf = np.zeros((e, cap, d), np.float32)
    ok = pos < cap
    buf[eid[ok], pos[ok]] = x[ok]
    out = np.empty((e, cap, d), np.float32)
    for ee in range(e):
        a = buf[ee] @ w1[ee].T
        g = a / (1.0 + np.exp(-a))
        u = buf[ee] @ w3[ee].T
        out[ee] = (g * u) @ w2[ee].T
    pos_c = np.minimum(pos, cap - 1)
    return out[eid, pos_c]


def kernel(w1, w2, w3, x, num_tokens_per_expert):
    w1 = np.asarray(w1, dtype=np.float32)
    w2 = np.asarray(w2, dtype=np.float32)
    w3 = np.asarray(w3, dtype=np.float32)
    x = np.asarray(x, dtype=np.float32)
    counts = np.asarray(num_tokens_per_expert).astype(np.int32)

    if not (x.shape == (T, D) and w1.shape == (E, H, D)
            and np.all(counts == TE)):
        return _reference_fallback(w1, w2, w3, x, counts)

    nc = _get_nc()
    in_maps = []
    for e in range(E):
        in_maps.append(
            _prep_core_inputs(x[e * TE:(e + 1) * TE], w1[e], w3[e], w2[e])
        )
    res = run_bass_kernel_spmd(nc, in_maps, list(range(E)))

    out = np.empty((T, D), dtype=np.float32)
    for e in range(E):
        y = res.results[e]["y"]  # [ID, 128, TE] bf16
        out[e * TE:(e + 1) * TE] = (
            y.astype(np.float32).transpose(2, 0, 1).reshape(TE, D)
        )
    return out


# revision 13
# speedup vs baseline: 1.0345x; 1.0009x over previous
"""Grouped SwiGLU MoE FFN (8 experts) on 8 Trainium2 NeuronCores.

Expert-parallel: core e owns expert e's weights and its contiguous slice of
tokens (inputs arrive pre-sorted by expert).  Per core we compute
    g = silu(x_e @ w1_e.T); u = x_e @ w3_e.T; y_e = (g*u) @ w2_e.T

Matmuls run on the PE array as fp8(e4m3) DoubleRow pairs at 0.5 cycles/row,
2x the bf16/fp32r rate.  To stay inside the 2e-2 error budget each operand
is split into hi+lo e4m3 parts (a "Karatsuba" split): for y = a@b we compute
a_hi@b_hi + a_lo@b_hi + a_hi@b_lo and drop the lo@lo term, giving ~bf16
accuracy at 0.75x the bf16 PE cost.  Each DoubleRow instruction packs two
independent 128-deep products: hi@hi terms pair adjacent contraction strips
(k, k+1); the two cross terms for one strip share one instruction via
stationary slot order (lo,hi) against moving slot order (hi,lo).

Weights are pre-scaled by 2^8 on the host so their hi parts sit in e4m3's
normal range; the scale is folded back in on the Act engine (silu input
scale and the final psum->bf16 copy).  All hi/lo splitting and layout
packing for x/w1/w3/w2 happens on the host in numpy; the device sees fp8
operands laid out partition-major and streams:
  phase A: per h-strip j, per 512-token tile: g,u psums -> silu (ACT),
           h = g*u (DVE) -> h_hi, h_lo fp8 tiles (DVE)
  phase B: per d-strip i: y psum over 12 padded h-strips -> bf16 out.
"""

import sys

sys.path.insert(0, "/opt/trn_rl_repo")

import numpy as np
import ml_dtypes

import concourse.bass as bass
import concourse.mybir as mybir
import concourse.tile as tile
from concourse import bacc
from concourse.bass import ts
from concourse.bass_utils import run_bass_kernel_spmd

F32 = mybir.dt.float32
BF16 = mybir.dt.bfloat16
F8 = mybir.dt.float8e4
NP_F8 = ml_dtypes.float8_e4m3fn
DR = mybir.MatmulPerfMode.DoubleRow
MULT = mybir.AluOpType.mult
SUBTRACT = mybir.AluOpType.subtract

E, H, D, T = 8, 1408, 2048, 16384
TE = T // E            # tokens per expert (uniform fast path)
KD = D // 128          # 16 contraction strips over d
JH = H // 128          # 11 h strips
JH2 = JH + 1           # padded to even for DoubleRow hi@hi pairing in mm2
ID = D // 128          # 16 output d strips
NT = TE // 512         # 4 token tiles of 512
WS = 256.0             # weight pre-scale (2^8) for e4m3 range health
WARMUP_N = 110         # dummy PE matmuls to cover startup DMA + pstate ramp


def _build_program():
    nc = bacc.Bacc("TRN2", target_bir_lowering=False, debug=False, num_devices=E)

    # [p, k, sl, t]: sl 0=hi, 1=lo of x[t, 128k+p]
    x_d = nc.dram_tensor("xq", [128, KD, 2, TE], F8, kind="ExternalInput").ap()
    # [p, j, s, k, l, m]: s 0=w1,1=w3; l 0=lo,1=hi of (WS*w)[128j+m, 128k+p]
    w13_d = nc.dram_tensor("w13q", [128, JH, 2, KD, 2, 128], F8,
                           kind="ExternalInput").ap()
    # [p, i, kk, l, m]: l 0=lo,1=hi of (WS*w2)[128i+m, 128kk+p]; kk=11 zero pad
    w2_d = nc.dram_tensor("w2q", [128, ID, JH2, 2, 128], F8,
                          kind="ExternalInput").ap()
    # [i, p, t] = y[t, 128i+p]
    y_d = nc.dram_tensor("y", [ID, 128, TE], BF16, kind="ExternalOutput").ap()

    with tile.TileContext(nc) as tc:
        with (
            tc.tile_pool(name="xp", bufs=1) as xp,
            tc.tile_pool(name="wp", bufs=5) as wp,
            tc.tile_pool(name="w2p", bufs=5) as w2p,
            tc.tile_pool(name="hp", bufs=1) as hp,
            tc.tile_pool(name="sp", bufs=2) as sp,
            tc.tile_pool(name="fp", bufs=2) as fp,
            tc.tile_pool(name="yp", bufs=2) as yp,
            tc.tile_pool(name="psA", bufs=2, space="PSUM") as psA,
            tc.tile_pool(name="psB", bufs=3, space="PSUM") as psB,
            tc.tile_pool(name="psW", bufs=1, space="PSUM") as psW,
        ):
            # h strips in fp8 hi/lo; strip JH (=11) is the zero pad for mm2
            # hi@hi pairing (its w2 slot is also zero, but the memset below
            # keeps any stale NaN encodings out of the pair).
            h = hp.tile([128, JH2, 2, TE], F8, tag="h")

            # PE warmup: dummy DoubleRow matmuls on a zeroed fp8 tile keep the
            # PE busy (and its pstate ramping toward full clock) while the
            # first weight/activation DMAs stream in.
            wu = sp.tile([128, 2, 128], F8, tag="wu", bufs=1)
            nc.vector.memset(wu[:], 0.0)
            pw = psW.tile([128, 128], F32, tag="pw")
            for _ in range(WARMUP_N):
                nc.tensor.matmul(pw[:], wu[:], wu[:], start=True, stop=True,
                                 perf_mode=DR)

            # startup DMAs, finest-needed-first so the first real matmuls can
            # begin as soon as possible: w1 strip 0 (g weights), x hi parts of
            # token tile 0, then w3 strip 0 (u weights), x lo, next strips.
            xt = xp.tile([128, KD, 2, TE], F8, tag="xt")
            w13_q = []

            def w13_fetch(j, split=False):
                t = wp.tile([128, 2, KD, 2, 128], F8, tag="w13", name="w13t")
                if split:
                    nc.sync.dma_start(t[:, 0], w13_d[:, j, 0])
                    nc.sync.dma_start(
                        xt[:, :, 0, ts(0, 512)], x_d[:, :, 0, ts(0, 512)]
                    )
                    nc.sync.dma_start(t[:, 1], w13_d[:, j, 1])
                    nc.sync.dma_start(
                        xt[:, :, 1, ts(0, 512)], x_d[:, :, 1, ts(0, 512)]
                    )
                else:
                    nc.sync.dma_start(t[:], w13_d[:, j])
                w13_q.append(t)

            w13_fetch(0, split=True)
            for j in (1, 2, 3):
                w13_fetch(j)
            # zero pad strip for mm2 (Pool engine; off the DMA path)
            nc.vector.memset(h[:, JH, 0, :], 0.0)

            def emit_block(w13, j, tt, hh_first):
                tsl = ts(tt, 512)
                pg = psA.tile([128, 512], F32, tag="pg")
                pu = psA.tile([128, 512], F32, tag="pu")
                order = ("hh", "cross") if hh_first else ("mix",)
                for phase in order:
                    for s, ps_ in ((0, pg), (1, pu)):
                        if phase in ("hh", "mix"):
                            # hi@hi over strip pairs (k, k+1)
                            for k in range(0, KD, 2):
                                nc.tensor.matmul(
                                    ps_[:], w13[:, s, k:k + 2, 1, :],
                                    xt[:, k:k + 2, 0, tsl],
                                    start=(k == 0), stop=False, perf_mode=DR,
                                )
                        if phase in ("cross", "mix"):
                            # cross terms: stationary (lo,hi) x moving (hi,lo)
                            for k in range(KD):
                                nc.tensor.matmul(
                                    ps_[:], w13[:, s, k, :, :],
                                    xt[:, k, :, tsl],
                                    start=False, stop=(k == KD - 1),
                                    perf_mode=DR,
                                )
                sg = sp.tile([128, 512], F32, tag="sg")
                nc.scalar.activation(
                    sg[:], pg[:], mybir.ActivationFunctionType.Silu,
                    scale=1.0 / WS,
                )
                hf = fp.tile([128, 512], F32, tag="hf")
                # hf = (pu * 1/WS) * sg = u * g
                nc.vector.scalar_tensor_tensor(
                    hf[:], pu[:], 1.0 / WS, sg[:], op0=MULT, op1=MULT
                )
                nc.vector.tensor_copy(h[:, j, 0, tsl], hf[:])
                nc.vector.tensor_sub(h[:, j, 1, tsl], hf[:], h[:, j, 0, tsl])

            # ---- phase A sweep 1: token tile 0 across all strips ----
            # x arrives while the w13 stream feeds the PE; later token tiles
            # stream in between weight strips.
            for j in range(JH):
                if j + 4 < JH:
                    w13_fetch(j + 4)
                emit_block(w13_q[j], j, 0, hh_first=True)
                if j == 0:
                    nc.sync.dma_start(
                        xt[:, :, :, ts(1, 512)], x_d[:, :, :, ts(1, 512)]
                    )
                elif j == 3:
                    nc.sync.dma_start(
                        xt[:, :, :, ts(2, 512)], x_d[:, :, :, ts(2, 512)]
                    )
                elif j == 6:
                    nc.sync.dma_start(
                        xt[:, :, :, ts(3, 512)], x_d[:, :, :, ts(3, 512)]
                    )

            # ---- phase A sweep 2: token tiles 1..3, w13 re-streamed ----
            w13_q.clear()
            w13_fetch(0)
            w13_fetch(1)
            # w2 prefetch (needed only in phase B)
            w2_tiles = []
            for i in range(4):
                w2t = w2p.tile([128, JH2, 2, 128], F8, tag="w2", name="w2t")
                nc.sync.dma_start(w2t[:], w2_d[:, i])
                w2_tiles.append(w2t)
            for j in range(JH):
                if j + 2 < JH:
                    w13_fetch(j + 2)
                for tt in range(1, NT):
                    emit_block(w13_q[j], j, tt, hh_first=False)

            # ---- phase B: y matmuls + store ----
            for i in range(ID):
                if i + 4 < ID:
                    w2t = w2p.tile([128, JH2, 2, 128], F8, tag="w2", name="w2t")
                    nc.sync.dma_start(w2t[:], w2_d[:, i + 4])
                    w2_tiles.append(w2t)
                w2 = w2_tiles[i]
                y_sb = yp.tile([128, TE], BF16, tag="ysb")
                last_i = i == ID - 1
                # the final token tile of the final strip is processed as two
                # 256-token psum groups so its copy/store overlap the
                # preceding matmuls, shrinking the post-last-matmul tail
                chunks = [(tt * 512, 512) for tt in range(NT)]
                if last_i:
                    chunks = chunks[:-1] + [((NT - 1) * 512, 256),
                                            ((NT - 1) * 512 + 256, 256)]
                for ci, (t0, tw) in enumerate(chunks):
                    tsl = ts(t0 // tw, tw)
                    py = psB.tile([128, 512], F32, tag="py")
                    for kk in range(0, JH2, 2):
                        nc.tensor.matmul(
                            py[:, :tw], w2[:, kk:kk + 2, 1, :],
                            h[:, kk:kk + 2, 0, tsl],
                            start=(kk == 0), stop=False, perf_mode=DR,
                        )
                    for kk in range(JH):
                        nc.tensor.matmul(
                            py[:, :tw], w2[:, kk, :, :], h[:, kk, :, tsl],
                            start=False, stop=(kk == JH - 1), perf_mode=DR,
                        )
                    nc.scalar.activation(
                        y_sb[:, tsl], py[:, :tw],
                        mybir.ActivationFunctionType.Copy, scale=1.0 / WS,
                    )
                    if last_i and ci == len(chunks) - 3:
                        # flush all but the final two 256-token chunks early
                        nc.sync.dma_start(
                            y_d[i, :, ts(0, 512 * (NT - 1))],
                            y_sb[:, ts(0, 512 * (NT - 1))],
                        )
                    elif last_i and ci >= len(chunks) - 2:
                        nc.sync.dma_start(y_d[i, :, tsl], y_sb[:, tsl])
                if not last_i:
                    nc.sync.dma_start(y_d[i], y_sb[:])

    nc.compile()
    return nc


_NC = None


def _get_nc():
    global _NC
    if _NC is None:
        _NC = _build_program()
    return _NC


def _hilo(a):
    hi = a.astype(NP_F8)
    lo = (a - hi.astype(np.float32)).astype(NP_F8)
    return hi, lo


def _prep_core_inputs(x_e, w1_e, w3_e, w2_e):
    # xq[p, k, sl, t] with sl=(hi,lo) of x_e[t, 128k+p]
    xr = np.ascontiguousarray(x_e.T).reshape(KD, 128, TE)   # [k, p, t]
    x_hi, x_lo = _hilo(xr)
    xq = np.stack([x_hi, x_lo], axis=2)                      # [k, p, sl, t]
    xq = np.ascontiguousarray(xq.transpose(1, 0, 2, 3))      # [p, k, sl, t]

    # w13q[p, j, s, k, l, m] with l=(lo,hi) of WS*w[128j+m, 128k+p]
    def pack_w13(w):
        r = (w * WS).reshape(JH, 128, KD, 128)               # [j, m, k, p]
        hi, lo = _hilo(r)
        t = np.stack([lo, hi], axis=3)                       # [j, m, k, l, p]
        return t.transpose(4, 0, 2, 3, 1)                    # [p, j, k, l, m]

    w13q = np.stack([pack_w13(w1_e), pack_w13(w3_e)], axis=2)  # [p,j,s,k,l,m]
    w13q = np.ascontiguousarray(w13q)

    # w2q[p, i, kk, l, m] with l=(lo,hi) of WS*w2[128i+m, 128kk+p]; pad kk=11
    r2 = (w2_e * WS).reshape(ID, 128, JH, 128)               # [i, m, kk, p]
    hi2, lo2 = _hilo(r2)
    t2 = np.stack([lo2, hi2], axis=3)                        # [i, m, kk, l, p]
    t2 = t2.transpose(4, 0, 2, 3, 1)                         # [p, i, kk, l, m]
    w2q = np.zeros((128, ID, JH2, 2, 128), dtype=NP_F8)
    w2q[:, :, :JH] = t2
    return {"xq": xq, "w13q": w13q, "w2q": np.ascontiguousarray(w2q)}


def _reference_fallback(w1, w2, w3, x, counts):
    # Exact numpy mirror of the jax reference (incl. scatter-drop / gather-clamp)
    e, h, d = w1.shape
    t = x.shape[0]
    cap = 2 * (t // e)
    counts = counts.astype(np.int64)
    offsets = np.concatenate([[0], np.cumsum(counts)[:-1]])
    eid = np.repeat(np.arange(e), counts)[:t]
    pos = np.arange(t) - offsets[eid]
    buf = np.zeros((e, cap, d), np.float32)
    ok = pos < cap
    buf[eid[ok], pos[ok]] = x[ok]
    out = np.empty((e, cap, d), np.float32)
    for ee in range(e):
        a = buf[ee] @ w1[ee].T
        g = a / (1.0 + np.exp(-a))
        u = buf[ee] @ w3[ee].T
        out[ee] = (g * u) @ w2[ee].T
    pos_c = np.minimum(pos, cap - 1)
    return out[eid, pos_c]


def kernel(w1, w2, w3, x, num_tokens_per_expert):
    w1 = np.asarray(w1, dtype=np.float32)
    w2 = np.asarray(w2, dtype=np.float32)
    w3 = np.asarray(w3, dtype=np.float32)
    x = np.asarray(x, dtype=np.float32)
    counts = np.asarray(num_tokens_per_expert).astype(np.int32)

    if not (x.shape == (T, D) and w1.shape == (E, H, D)
            and np.all(counts == TE)):
        return _reference_fallback(w1, w2, w3, x, counts)

    nc = _get_nc()
    in_maps = []
    for e in range(E):
        in_maps.append(
            _prep_core_inputs(x[e * TE:(e + 1) * TE], w1[e], w3[e], w2[e])
        )
    res = run_bass_kernel_spmd(nc, in_maps, list(range(E)))

    out = np.empty((T, D), dtype=np.float32)
    for e in range(E):
        y = res.results[e]["y"]  # [ID, 128, TE] bf16
        out[e * TE:(e + 1) * TE] = (
            y.astype(np.float32).transpose(2, 0, 1).reshape(TE, D)
        )
    return out


# revision 15
# speedup vs baseline: 1.0362x; 1.0016x over previous
"""Grouped SwiGLU MoE FFN (8 experts) on 8 Trainium2 NeuronCores.

Expert-parallel: core e owns expert e's weights and its contiguous slice of
tokens (inputs arrive pre-sorted by expert).  Per core we compute
    g = silu(x_e @ w1_e.T); u = x_e @ w3_e.T; y_e = (g*u) @ w2_e.T

Matmuls run on the PE array as fp8(e4m3) DoubleRow pairs at 0.5 cycles/row,
2x the bf16/fp32r rate.  To stay inside the 2e-2 error budget each operand
is split into hi+lo e4m3 parts (a "Karatsuba" split): for y = a@b we compute
a_hi@b_hi + a_lo@b_hi + a_hi@b_lo and drop the lo@lo term, giving ~bf16
accuracy at 0.75x the bf16 PE cost.  Each DoubleRow instruction packs two
independent 128-deep products: hi@hi terms pair adjacent contraction strips
(k, k+1); the two cross terms for one strip share one instruction via
stationary slot order (lo,hi) against moving slot order (hi,lo).

Weights are pre-scaled by 2^8 on the host so their hi parts sit in e4m3's
normal range; the scale is folded back in on the Act engine (silu input
scale and the final psum->bf16 copy).  All hi/lo splitting and layout
packing for x/w1/w3/w2 happens on the host in numpy; the device sees fp8
operands laid out partition-major and streams:
  phase A: per h-strip j, per 512-token tile: g,u psums -> silu (ACT),
           h = g*u (DVE) -> h_hi, h_lo fp8 tiles (DVE)
  phase B: per d-strip i: y psum over 12 padded h-strips -> bf16 out.
"""

import sys

sys.path.insert(0, "/opt/trn_rl_repo")

import numpy as np
import ml_dtypes

import concourse.bass as bass
import concourse.mybir as mybir
import concourse.tile as tile
from concourse import bacc
from concourse.bass import ts
from concourse.bass_utils import run_bass_kernel_spmd

F32 = mybir.dt.float32
BF16 = mybir.dt.bfloat16
F8 = mybir.dt.float8e4
NP_F8 = ml_dtypes.float8_e4m3fn
DR = mybir.MatmulPerfMode.DoubleRow
MULT = mybir.AluOpType.mult
SUBTRACT = mybir.AluOpType.subtract

E, H, D, T = 8, 1408, 2048, 16384
TE = T // E            # tokens per expert (uniform fast path)
KD = D // 128          # 16 contraction strips over d
JH = H // 128          # 11 h strips
JH2 = JH + 1           # padded to even for DoubleRow hi@hi pairing in mm2
ID = D // 128          # 16 output d strips
NT = TE // 512         # 4 token tiles of 512
WS = 256.0             # weight pre-scale (2^8) for e4m3 range health
WARMUP_N = 110         # dummy PE matmuls to cover startup DMA + pstate ramp


def _build_program():
    nc = bacc.Bacc("TRN2", target_bir_lowering=False, debug=False, num_devices=E)

    # [p, k, sl, t]: sl 0=hi, 1=lo of x[t, 128k+p]
    x_d = nc.dram_tensor("xq", [128, KD, 2, TE], F8, kind="ExternalInput").ap()
    # [p, j, s, k, l, m]: s 0=w1,1=w3; l 0=lo,1=hi of (WS*w)[128j+m, 128k+p]
    w13_d = nc.dram_tensor("w13q", [128, JH, 2, KD, 2, 128], F8,
                           kind="ExternalInput").ap()
    # [p, i, kk, l, m]: l 0=lo,1=hi of (WS*w2)[128i+m, 128kk+p]; kk=11 zero pad
    w2_d = nc.dram_tensor("w2q", [128, ID, JH2, 2, 128], F8,
                          kind="ExternalInput").ap()
    # [i, p, t] = y[t, 128i+p]
    y_d = nc.dram_tensor("y", [ID, 128, TE], BF16, kind="ExternalOutput").ap()

    with tile.TileContext(nc) as tc:
        with (
            tc.tile_pool(name="xp", bufs=1) as xp,
            tc.tile_pool(name="wp", bufs=5) as wp,
            tc.tile_pool(name="w2p", bufs=5) as w2p,
            tc.tile_pool(name="hp", bufs=1) as hp,
            tc.tile_pool(name="sp", bufs=2) as sp,
            tc.tile_pool(name="fp", bufs=2) as fp,
            tc.tile_pool(name="yp", bufs=2) as yp,
            tc.tile_pool(name="psA", bufs=2, space="PSUM") as psA,
            tc.tile_pool(name="psB", bufs=3, space="PSUM") as psB,
            tc.tile_pool(name="psW", bufs=1, space="PSUM") as psW,
        ):
            # h strips in fp8 hi/lo; strip JH (=11) is the zero pad for mm2
            # hi@hi pairing (its w2 slot is also zero, but the memset below
            # keeps any stale NaN encodings out of the pair).
            h = hp.tile([128, JH2, 2, TE], F8, tag="h")

            # PE warmup: dummy DoubleRow matmuls on a zeroed fp8 tile keep the
            # PE busy (and its pstate ramping toward full clock) while the
            # first weight/activation DMAs stream in.
            wu = sp.tile([128, 2, 128], F8, tag="wu", bufs=1)
            nc.vector.memset(wu[:], 0.0)
            pw = psW.tile([128, 128], F32, tag="pw")
            for _ in range(WARMUP_N):
                nc.tensor.matmul(pw[:], wu[:], wu[:], start=True, stop=True,
                                 perf_mode=DR)

            # startup DMAs, finest-needed-first so the first real matmuls can
            # begin as soon as possible: w1 strip 0 (g weights), x hi parts of
            # token tile 0, then w3 strip 0 (u weights), x lo, next strips.
            xt = xp.tile([128, KD, 2, TE], F8, tag="xt")
            w13_q = []

            def w13_fetch(j, split=False):
                t = wp.tile([128, 2, KD, 2, 128], F8, tag="w13", name="w13t")
                if split:
                    nc.sync.dma_start(t[:, 0], w13_d[:, j, 0])
                    nc.sync.dma_start(
                        xt[:, :, 0, ts(0, 512)], x_d[:, :, 0, ts(0, 512)]
                    )
                    nc.sync.dma_start(t[:, 1], w13_d[:, j, 1])
                    # x lo parts in two k-halves so the first cross matmuls
                    # can start while the second half streams
                    nc.sync.dma_start(
                        xt[:, :KD // 2, 1, ts(0, 512)],
                        x_d[:, :KD // 2, 1, ts(0, 512)],
                    )
                    nc.sync.dma_start(
                        xt[:, KD // 2:, 1, ts(0, 512)],
                        x_d[:, KD // 2:, 1, ts(0, 512)],
                    )
                else:
                    nc.sync.dma_start(t[:], w13_d[:, j])
                w13_q.append(t)

            w13_fetch(0, split=True)
            for j in (1, 2, 3):
                w13_fetch(j)
            # zero pad strip for mm2 (Pool engine; off the DMA path)
            nc.vector.memset(h[:, JH, 0, :], 0.0)

            def emit_block(w13, j, tt, hh_first):
                tsl = ts(tt, 512)
                pg = psA.tile([128, 512], F32, tag="pg")
                pu = psA.tile([128, 512], F32, tag="pu")
                # hh: hi@hi over strip pairs (k, k+1)
                # cross: stationary (lo,hi) x moving (hi,lo), split in
                # k-halves when hh_first so j0 tracks the split x-lo DMAs
                if hh_first:
                    phases = [("hh", range(0, KD, 2)),
                              ("cross", range(0, KD // 2)),
                              ("cross", range(KD // 2, KD))]
                else:
                    phases = [("hh", range(0, KD, 2)), ("cross", range(KD))]
                for phase, krange in phases:
                    for s, ps_ in ((0, pg), (1, pu)):
                        for k in krange:
                            if phase == "hh":
                                nc.tensor.matmul(
                                    ps_[:], w13[:, s, k:k + 2, 1, :],
                                    xt[:, k:k + 2, 0, tsl],
                                    start=(k == 0), stop=False, perf_mode=DR,
                                )
                            else:
                                nc.tensor.matmul(
                                    ps_[:], w13[:, s, k, :, :],
                                    xt[:, k, :, tsl],
                                    start=False, stop=(k == KD - 1),
                                    perf_mode=DR,
                                )
                sg = sp.tile([128, 512], F32, tag="sg")
                nc.scalar.activation(
                    sg[:], pg[:], mybir.ActivationFunctionType.Silu,
                    scale=1.0 / WS,
                )
                hf = fp.tile([128, 512], F32, tag="hf")
                # hf = (pu * 1/WS) * sg = u * g
                nc.vector.scalar_tensor_tensor(
                    hf[:], pu[:], 1.0 / WS, sg[:], op0=MULT, op1=MULT
                )
                nc.vector.tensor_copy(h[:, j, 0, tsl], hf[:])
                nc.vector.tensor_sub(h[:, j, 1, tsl], hf[:], h[:, j, 0, tsl])

            # ---- phase A sweep 1: token tile 0 across all strips ----
            # x arrives while the w13 stream feeds the PE; later token tiles
            # stream in between weight strips.
            for j in range(JH):
                if j + 4 < JH:
                    w13_fetch(j + 4)
                emit_block(w13_q[j], j, 0, hh_first=True)
                if j == 0:
                    nc.sync.dma_start(
                        xt[:, :, :, ts(1, 512)], x_d[:, :, :, ts(1, 512)]
                    )
                elif j == 3:
                    nc.sync.dma_start(
                        xt[:, :, :, ts(2, 512)], x_d[:, :, :, ts(2, 512)]
                    )
                elif j == 6:
                    nc.sync.dma_start(
                        xt[:, :, :, ts(3, 512)], x_d[:, :, :, ts(3, 512)]
                    )

            # ---- phase A sweep 2: token tiles 1..3, w13 re-streamed ----
            w13_q.clear()
            w13_fetch(0)
            w13_fetch(1)
            # w2 prefetch (needed only in phase B)
            w2_tiles = []
            for i in range(4):
                w2t = w2p.tile([128, JH2, 2, 128], F8, tag="w2", name="w2t")
                nc.sync.dma_start(w2t[:], w2_d[:, i])
                w2_tiles.append(w2t)
            for j in range(JH):
                if j + 2 < JH:
                    w13_fetch(j + 2)
                for tt in range(1, NT):
                    emit_block(w13_q[j], j, tt, hh_first=False)

            # ---- phase B: y matmuls + store ----
            for i in range(ID):
                if i + 4 < ID:
                    w2t = w2p.tile([128, JH2, 2, 128], F8, tag="w2", name="w2t")
                    nc.sync.dma_start(w2t[:], w2_d[:, i + 4])
                    w2_tiles.append(w2t)
                w2 = w2_tiles[i]
                y_sb = yp.tile([128, TE], BF16, tag="ysb")
                last_i = i == ID - 1
                # the final token tile of the final strip is processed as two
                # 256-token psum groups so its copy/store overlap the
                # preceding matmuls, shrinking the post-last-matmul tail
                chunks = [(tt * 512, 512) for tt in range(NT)]
                if last_i:
                    chunks = chunks[:-1] + [((NT - 1) * 512, 256),
                                            ((NT - 1) * 512 + 256, 256)]
                for ci, (t0, tw) in enumerate(chunks):
                    tsl = ts(t0 // tw, tw)
                    py = psB.tile([128, 512], F32, tag="py")
                    for kk in range(0, JH2, 2):
                        nc.tensor.matmul(
                            py[:, :tw], w2[:, kk:kk + 2, 1, :],
                            h[:, kk:kk + 2, 0, tsl],
                            start=(kk == 0), stop=False, perf_mode=DR,
                        )
                    for kk in range(JH):
                        nc.tensor.matmul(
                            py[:, :tw], w2[:, kk, :, :], h[:, kk, :, tsl],
                            start=False, stop=(kk == JH - 1), perf_mode=DR,
                        )
                    nc.scalar.activation(
                        y_sb[:, tsl], py[:, :tw],
                        mybir.ActivationFunctionType.Copy, scale=1.0 / WS,
                    )
                    if last_i and ci == len(chunks) - 3:
                        # flush all but the final two 256-token chunks early
                        nc.sync.dma_start(
                            y_d[i, :, ts(0, 512 * (NT - 1))],
                            y_sb[:, ts(0, 512 * (NT - 1))],
                        )
                    elif last_i and ci >= len(chunks) - 2:
                        nc.sync.dma_start(y_d[i, :, tsl], y_sb[:, tsl])
                if not last_i:
                    nc.sync.dma_start(y_d[i], y_sb[:])

    nc.compile()
    return nc


_NC = None


def _get_nc():
    global _NC
    if _NC is None:
        _NC = _build_program()
    return _NC


def _hilo(a):
    hi = a.astype(NP_F8)
    lo = (a - hi.astype(np.float32)).astype(NP_F8)
    return hi, lo


def _prep_core_inputs(x_e, w1_e, w3_e, w2_e):
    # xq[p, k, sl, t] with sl=(hi,lo) of x_e[t, 128k+p]
    xr = np.ascontiguousarray(x_e.T).reshape(KD, 128, TE)   # [k, p, t]
    x_hi, x_lo = _hilo(xr)
    xq = np.stack([x_hi, x_lo], axis=2)                      # [k, p, sl, t]
    xq = np.ascontiguousarray(xq.transpose(1, 0, 2, 3))      # [p, k, sl, t]

    # w13q[p, j, s, k, l, m] with l=(lo,hi) of WS*w[128j+m, 128k+p]
    def pack_w13(w):
        r = (w * WS).reshape(JH, 128, KD, 128)               # [j, m, k, p]
        hi, lo = _hilo(r)
        t = np.stack([lo, hi], axis=3)                       # [j, m, k, l, p]
        return t.transpose(4, 0, 2, 3, 1)                    # [p, j, k, l, m]

    w13q = np.stack([pack_w13(w1_e), pack_w13(w3_e)], axis=2)  # [p,j,s,k,l,m]
    w13q = np.ascontiguousarray(w13q)

    # w2q[p, i, kk, l, m] with l=(lo,hi) of WS*w2[128i+m, 128kk+p]; pad kk=11
    r2 = (w2_e * WS).reshape(ID, 128, JH, 128)               # [i, m, kk, p]
    hi2, lo2 = _hilo(r2)
    t2 = np.stack([lo2, hi2], axis=3)                        # [i, m, kk, l, p]
    t2 = t2.transpose(4, 0, 2, 3, 1)                         # [p, i, kk, l, m]
    w2q = np.zeros((128, ID, JH2, 2, 128), dtype=NP_F8)
    w2q[:, :, :JH] = t2
    return {"xq": xq, "w13q": w13q, "w2q": np.ascontiguousarray(w2q)}


def _reference_fallback(w1, w2, w3, x, counts):
    # Exact numpy mirror of the jax reference (incl. scatter-drop / gather-clamp)
    e, h, d = w1.shape
    t = x.shape[0]
    cap = 2 * (t // e)
    counts = counts.astype(np.int64)
    offsets = np.concatenate([[0], np.cumsum(counts)[:-1]])
    eid = np.repeat(np.arange(e), counts)[:t]
    pos = np.arange(t) - offsets[eid]
    buf = np.zeros((e, cap, d), np.float32)
    ok = pos < cap
    buf[eid[ok], pos[ok]] = x[ok]
    out = np.empty((e, cap, d), np.float32)
    for ee in range(e):
        a = buf[ee] @ w1[ee].T
        g = a / (1.0 + np.exp(-a))
        u = buf[ee] @ w3[ee].T
        out[ee] = (g * u) @ w2[ee].T
    pos_c = np.minimum(pos, cap - 1)
    return out[eid, pos_c]


def kernel(w1, w2, w3, x, num_tokens_per_expert):
    w1 = np.asarray(w1, dtype=np.float32)
    w2 = np.asarray(w2, dtype=np.float32)
    w3 = np.asarray(w3, dtype=np.float32)
    x = np.asarray(x, dtype=np.float32)
    counts = np.asarray(num_tokens_per_expert).astype(np.int32)

    if not (x.shape == (T, D) and w1.shape == (E, H, D)
            and np.all(counts == TE)):
        return _reference_fallback(w1, w2, w3, x, counts)

    nc = _get_nc()
    in_maps = []
    for e in range(E):
        in_maps.append(
            _prep_core_inputs(x[e * TE:(e + 1) * TE], w1[e], w3[e], w2[e])
        )
    res = run_bass_kernel_spmd(nc, in_maps, list(range(E)))

    out = np.empty((T, D), dtype=np.float32)
    for e in range(E):
        y = res.results[e]["y"]  # [ID, 128, TE] bf16
        out[e * TE:(e + 1) * TE] = (
            y.astype(np.float32).transpose(2, 0, 1).reshape(TE, D)
        )
    return out


# revision 17
# speedup vs baseline: 1.0637x; 1.0266x over previous
"""Grouped SwiGLU MoE FFN (8 experts) on 8 Trainium2 NeuronCores.

Expert-parallel: core e owns expert e's weights and its contiguous slice of
tokens (inputs arrive pre-sorted by expert).  Per core we compute
    g = silu(x_e @ w1_e.T); u = x_e @ w3_e.T; y_e = (g*u) @ w2_e.T

Matmuls run on the PE array as fp8(e4m3) DoubleRow pairs at 0.5 cycles/row,
2x the bf16/fp32r rate.  To stay inside the 2e-2 error budget each operand
is split into hi+lo e4m3 parts (a "Karatsuba" split): for y = a@b we compute
a_hi@b_hi + a_lo@b_hi + a_hi@b_lo and drop the lo@lo term, giving ~bf16
accuracy at 0.75x the bf16 PE cost.  Each DoubleRow instruction packs two
independent 128-deep products: hi@hi terms pair adjacent contraction strips
(k, k+1); the two cross terms for one strip share one instruction via
stationary slot order (lo,hi) against moving slot order (hi,lo).

Weights are pre-scaled by 2^8 on the host so their hi parts sit in e4m3's
normal range; the scale is folded back in on the Act engine (silu input
scale and the final psum->bf16 copy).  All hi/lo splitting and layout
packing for x/w1/w3/w2 happens on the host in numpy; the device sees fp8
operands laid out partition-major and streams:
  phase A: per h-strip j, per 512-token tile: g,u psums -> silu (ACT),
           h = g*u (DVE) -> h_hi, h_lo fp8 tiles (DVE)
  phase B: per d-strip i: y psum over 12 padded h-strips -> bf16 out.
"""

import sys

sys.path.insert(0, "/opt/trn_rl_repo")

import numpy as np
import ml_dtypes

import concourse.bass as bass
import concourse.mybir as mybir
import concourse.tile as tile
from concourse import bacc
from concourse.bass import ts
from concourse.bass_utils import run_bass_kernel_spmd

F32 = mybir.dt.float32
BF16 = mybir.dt.bfloat16
F8 = mybir.dt.float8e4
NP_F8 = ml_dtypes.float8_e4m3fn
DR = mybir.MatmulPerfMode.DoubleRow
MULT = mybir.AluOpType.mult
SUBTRACT = mybir.AluOpType.subtract

E, H, D, T = 8, 1408, 2048, 16384
TE = T // E            # tokens per expert (uniform fast path)
KD = D // 128          # 16 contraction strips over d
JH = H // 128          # 11 h strips
JH2 = JH + 1           # padded to even for DoubleRow hi@hi pairing in mm2
ID = D // 128          # 16 output d strips
NT = TE // 512         # 4 token tiles of 512
WS = 256.0             # weight pre-scale (2^8) for e4m3 range health
WARMUP_N = 110         # dummy PE matmuls to cover startup DMA + pstate ramp
# spend error budget: drop the hi/lo cross correction on the last N
# contraction strips of the g / u matmuls (4.7us saved per strip; the
# hi@hi term still covers every strip)
DROP_G = 1
DROP_U = 1


def _build_program():
    nc = bacc.Bacc("TRN2", target_bir_lowering=False, debug=False, num_devices=E)

    # [p, k, sl, t]: sl 0=hi, 1=lo of x[t, 128k+p]
    x_d = nc.dram_tensor("xq", [128, KD, 2, TE], F8, kind="ExternalInput").ap()
    # [p, j, s, k, l, m]: s 0=w1,1=w3; l 0=lo,1=hi of (WS*w)[128j+m, 128k+p]
    w13_d = nc.dram_tensor("w13q", [128, JH, 2, KD, 2, 128], F8,
                           kind="ExternalInput").ap()
    # [p, i, kk, l, m]: l 0=lo,1=hi of (WS*w2)[128i+m, 128kk+p]; kk=11 zero pad
    w2_d = nc.dram_tensor("w2q", [128, ID, JH2, 2, 128], F8,
                          kind="ExternalInput").ap()
    # [i, p, t] = y[t, 128i+p]
    y_d = nc.dram_tensor("y", [ID, 128, TE], BF16, kind="ExternalOutput").ap()

    with tile.TileContext(nc) as tc:
        with (
            tc.tile_pool(name="xp", bufs=1) as xp,
            tc.tile_pool(name="wp", bufs=5) as wp,
            tc.tile_pool(name="w2p", bufs=5) as w2p,
            tc.tile_pool(name="hp", bufs=1) as hp,
            tc.tile_pool(name="sp", bufs=2) as sp,
            tc.tile_pool(name="fp", bufs=2) as fp,
            tc.tile_pool(name="yp", bufs=2) as yp,
            tc.tile_pool(name="psA", bufs=2, space="PSUM") as psA,
            tc.tile_pool(name="psB", bufs=3, space="PSUM") as psB,
            tc.tile_pool(name="psW", bufs=1, space="PSUM") as psW,
        ):
            # h strips in fp8 hi/lo; strip JH (=11) is the zero pad for mm2
            # hi@hi pairing (its w2 slot is also zero, but the memset below
            # keeps any stale NaN encodings out of the pair).
            h = hp.tile([128, JH2, 2, TE], F8, tag="h")

            # PE warmup: dummy DoubleRow matmuls on a zeroed fp8 tile keep the
            # PE busy (and its pstate ramping toward full clock) while the
            # first weight/activation DMAs stream in.
            wu = sp.tile([128, 2, 128], F8, tag="wu", bufs=1)
            nc.vector.memset(wu[:], 0.0)
            pw = psW.tile([128, 128], F32, tag="pw")
            for _ in range(WARMUP_N):
                nc.tensor.matmul(pw[:], wu[:], wu[:], start=True, stop=True,
                                 perf_mode=DR)

            # startup DMAs, finest-needed-first so the first real matmuls can
            # begin as soon as possible: w1 strip 0 (g weights), x hi parts of
            # token tile 0, then w3 strip 0 (u weights), x lo, next strips.
            xt = xp.tile([128, KD, 2, TE], F8, tag="xt")
            w13_q = []

            def w13_fetch(j, split=False):
                t = wp.tile([128, 2, KD, 2, 128], F8, tag="w13", name="w13t")
                if split:
                    nc.sync.dma_start(t[:, 0], w13_d[:, j, 0])
                    nc.sync.dma_start(
                        xt[:, :, 0, ts(0, 512)], x_d[:, :, 0, ts(0, 512)]
                    )
                    nc.sync.dma_start(t[:, 1], w13_d[:, j, 1])
                    # x lo parts in two k-halves so the first cross matmuls
                    # can start while the second half streams
                    nc.sync.dma_start(
                        xt[:, :KD // 2, 1, ts(0, 512)],
                        x_d[:, :KD // 2, 1, ts(0, 512)],
                    )
                    nc.sync.dma_start(
                        xt[:, KD // 2:, 1, ts(0, 512)],
                        x_d[:, KD // 2:, 1, ts(0, 512)],
                    )
                else:
                    nc.sync.dma_start(t[:], w13_d[:, j])
                w13_q.append(t)

            w13_fetch(0, split=True)
            for j in (1, 2, 3):
                w13_fetch(j)
            # zero pad strip for mm2 (Pool engine; off the DMA path)
            nc.vector.memset(h[:, JH, 0, :], 0.0)

            def emit_block(w13, j, tt, hh_first):
                tsl = ts(tt, 512)
                pg = psA.tile([128, 512], F32, tag="pg")
                pu = psA.tile([128, 512], F32, tag="pu")
                # hh: hi@hi over strip pairs (k, k+1)
                # cross: stationary (lo,hi) x moving (hi,lo), split in
                # k-halves when hh_first so j0 tracks the split x-lo DMAs
                klast = {0: KD - 1 - DROP_G, 1: KD - 1 - DROP_U}
                if hh_first:
                    phases = [("hh", range(0, KD, 2)),
                              ("cross", range(0, KD // 2)),
                              ("cross", range(KD // 2, KD))]
                else:
                    phases = [("hh", range(0, KD, 2)), ("cross", range(KD))]
                for phase, krange in phases:
                    for s, ps_ in ((0, pg), (1, pu)):
                        for k in krange:
                            if phase == "hh":
                                nc.tensor.matmul(
                                    ps_[:], w13[:, s, k:k + 2, 1, :],
                                    xt[:, k:k + 2, 0, tsl],
                                    start=(k == 0), stop=False, perf_mode=DR,
                                )
                            elif k <= klast[s]:
                                nc.tensor.matmul(
                                    ps_[:], w13[:, s, k, :, :],
                                    xt[:, k, :, tsl],
                                    start=False, stop=(k == klast[s]),
                                    perf_mode=DR,
                                )
                sg = sp.tile([128, 512], F32, tag="sg")
                nc.scalar.activation(
                    sg[:], pg[:], mybir.ActivationFunctionType.Silu,
                    scale=1.0 / WS,
                )
                hf = fp.tile([128, 512], F32, tag="hf")
                # hf = (pu * 1/WS) * sg = u * g
                nc.vector.scalar_tensor_tensor(
                    hf[:], pu[:], 1.0 / WS, sg[:], op0=MULT, op1=MULT
                )
                nc.vector.tensor_copy(h[:, j, 0, tsl], hf[:])
                nc.vector.tensor_sub(h[:, j, 1, tsl], hf[:], h[:, j, 0, tsl])

            # ---- phase A sweep 1: token tile 0 across all strips ----
            # x arrives while the w13 stream feeds the PE; later token tiles
            # stream in between weight strips.
            for j in range(JH):
                if j + 4 < JH:
                    w13_fetch(j + 4)
                emit_block(w13_q[j], j, 0, hh_first=True)
                if j == 0:
                    nc.sync.dma_start(
                        xt[:, :, :, ts(1, 512)], x_d[:, :, :, ts(1, 512)]
                    )
                elif j == 3:
                    nc.sync.dma_start(
                        xt[:, :, :, ts(2, 512)], x_d[:, :, :, ts(2, 512)]
                    )
                elif j == 6:
                    nc.sync.dma_start(
                        xt[:, :, :, ts(3, 512)], x_d[:, :, :, ts(3, 512)]
                    )

            # ---- phase A sweep 2: token tiles 1..3, w13 re-streamed ----
            w13_q.clear()
            w13_fetch(0)
            w13_fetch(1)
            # w2 prefetch (needed only in phase B)
            w2_tiles = []
            for i in range(4):
                w2t = w2p.tile([128, JH2, 2, 128], F8, tag="w2", name="w2t")
                nc.sync.dma_start(w2t[:], w2_d[:, i])
                w2_tiles.append(w2t)
            for j in range(JH):
                if j + 2 < JH:
                    w13_fetch(j + 2)
                for tt in range(1, NT):
                    emit_block(w13_q[j], j, tt, hh_first=False)

            # ---- phase B: y matmuls + store ----
            for i in range(ID):
                if i + 4 < ID:
                    w2t = w2p.tile([128, JH2, 2, 128], F8, tag="w2", name="w2t")
                    nc.sync.dma_start(w2t[:], w2_d[:, i + 4])
                    w2_tiles.append(w2t)
                w2 = w2_tiles[i]
                y_sb = yp.tile([128, TE], BF16, tag="ysb")
                last_i = i == ID - 1
                # the final token tile of the final strip is processed as two
                # 256-token psum groups so its copy/store overlap the
                # preceding matmuls, shrinking the post-last-matmul tail
                chunks = [(tt * 512, 512) for tt in range(NT)]
                if last_i:
                    chunks = chunks[:-1] + [((NT - 1) * 512, 256),
                                            ((NT - 1) * 512 + 256, 256)]
                for ci, (t0, tw) in enumerate(chunks):
                    tsl = ts(t0 // tw, tw)
                    py = psB.tile([128, 512], F32, tag="py")
                    for kk in range(0, JH2, 2):
                        nc.tensor.matmul(
                            py[:, :tw], w2[:, kk:kk + 2, 1, :],
                            h[:, kk:kk + 2, 0, tsl],
                            start=(kk == 0), stop=False, perf_mode=DR,
                        )
                    for kk in range(JH):
                        nc.tensor.matmul(
                            py[:, :tw], w2[:, kk, :, :], h[:, kk, :, tsl],
                            start=False, stop=(kk == JH - 1), perf_mode=DR,
                        )
                    nc.scalar.activation(
                        y_sb[:, tsl], py[:, :tw],
                        mybir.ActivationFunctionType.Copy, scale=1.0 / WS,
                    )
                    if last_i and ci == len(chunks) - 3:
                        # flush all but the final two 256-token chunks early
                        nc.sync.dma_start(
                            y_d[i, :, ts(0, 512 * (NT - 1))],
                            y_sb[:, ts(0, 512 * (NT - 1))],
                        )
                    elif last_i and ci >= len(chunks) - 2:
                        nc.sync.dma_start(y_d[i, :, tsl], y_sb[:, tsl])
                if not last_i:
                    nc.sync.dma_start(y_d[i], y_sb[:])

    nc.compile()
    return nc


_NC = None


def _get_nc():
    global _NC
    if _NC is None:
        _NC = _build_program()
    return _NC


def _hilo(a):
    hi = a.astype(NP_F8)
    lo = (a - hi.astype(np.float32)).astype(NP_F8)
    return hi, lo


def _prep_core_inputs(x_e, w1_e, w3_e, w2_e):
    # xq[p, k, sl, t] with sl=(hi,lo) of x_e[t, 128k+p]
    xr = np.ascontiguousarray(x_e.T).reshape(KD, 128, TE)   # [k, p, t]
    x_hi, x_lo = _hilo(xr)
    xq = np.stack([x_hi, x_lo], axis=2)                      # [k, p, sl, t]
    xq = np.ascontiguousarray(xq.transpose(1, 0, 2, 3))      # [p, k, sl, t]

    # w13q[p, j, s, k, l, m] with l=(lo,hi) of WS*w[128j+m, 128k+p]
    def pack_w13(w):
        r = (w * WS).reshape(JH, 128, KD, 128)               # [j, m, k, p]
        hi, lo = _hilo(r)
        t = np.stack([lo, hi], axis=3)                       # [j, m, k, l, p]
        return t.transpose(4, 0, 2, 3, 1)                    # [p, j, k, l, m]

    w13q = np.stack([pack_w13(w1_e), pack_w13(w3_e)], axis=2)  # [p,j,s,k,l,m]
    w13q = np.ascontiguousarray(w13q)

    # w2q[p, i, kk, l, m] with l=(lo,hi) of WS*w2[128i+m, 128kk+p]; pad kk=11
    r2 = (w2_e * WS).reshape(ID, 128, JH, 128)               # [i, m, kk, p]
    hi2, lo2 = _hilo(r2)
    t2 = np.stack([lo2, hi2], axis=3)                        # [i, m, kk, l, p]
    t2 = t2.transpose(4, 0, 2, 3, 1)                         # [p, i, kk, l, m]
    w2q = np.zeros((128, ID, JH2, 2, 128), dtype=NP_F8)
    w2q[:, :, :JH] = t2
    return {"xq": xq, "w13q": w13q, "w2q": np.ascontiguousarray(w2q)}


def _reference_fallback(w1, w2, w3, x, counts):
    # Exact numpy mirror of the jax reference (incl. scatter-drop / gather-clamp)
    e, h, d = w1.shape
    t = x.shape[0]
    cap = 2 * (t // e)
    counts = counts.astype(np.int64)
    offsets = np.concatenate([[0], np.cumsum(counts)[:-1]])
    eid = np.repeat(np.arange(e), counts)[:t]
    pos = np.arange(t) - offsets[eid]
    buf = np.zeros((e, cap, d), np.float32)
    ok = pos < cap
    buf[eid[ok], pos[ok]] = x[ok]
    out = np.empty((e, cap, d), np.float32)
    for ee in range(e):
        a = buf[ee] @ w1[ee].T
        g = a / (1.0 + np.exp(-a))
        u = buf[ee] @ w3[ee].T
        out[ee] = (g * u) @ w2[ee].T
    pos_c = np.minimum(pos, cap - 1)
    return out[eid, pos_c]


def kernel(w1, w2, w3, x, num_tokens_per_expert):
    w1 = np.asarray(w1, dtype=np.float32)
    w2 = np.asarray(w2, dtype=np.float32)
    w3 = np.asarray(w3, dtype=np.float32)
    x = np.asarray(x, dtype=np.float32)
    counts = np.asarray(num_tokens_per_expert).astype(np.int32)

    if not (x.shape == (T, D) and w1.shape == (E, H, D)
            and np.all(counts == TE)):
        return _reference_fallback(w1, w2, w3, x, counts)

    nc = _get_nc()
    in_maps = []
    for e in range(E):
        in_maps.append(
            _prep_core_inputs(x[e * TE:(e + 1) * TE], w1[e], w3[e], w2[e])
        )
    res = run_bass_kernel_spmd(nc, in_maps, list(range(E)))

    out = np.empty((T, D), dtype=np.float32)
    for e in range(E):
        y = res.results[e]["y"]  # [ID, 128, TE] bf16
        out[e * TE:(e + 1) * TE] = (
            y.astype(np.float32).transpose(2, 0, 1).reshape(TE, D)
        )
    return out


# revision 18
# speedup vs baseline: 1.0927x; 1.0273x over previous
"""Grouped SwiGLU MoE FFN (8 experts) on 8 Trainium2 NeuronCores.

Expert-parallel: core e owns expert e's weights and its contiguous slice of
tokens (inputs arrive pre-sorted by expert).  Per core we compute
    g = silu(x_e @ w1_e.T); u = x_e @ w3_e.T; y_e = (g*u) @ w2_e.T

Matmuls run on the PE array as fp8(e4m3) DoubleRow pairs at 0.5 cycles/row,
2x the bf16/fp32r rate.  To stay inside the 2e-2 error budget each operand
is split into hi+lo e4m3 parts (a "Karatsuba" split): for y = a@b we compute
a_hi@b_hi + a_lo@b_hi + a_hi@b_lo and drop the lo@lo term, giving ~bf16
accuracy at 0.75x the bf16 PE cost.  Each DoubleRow instruction packs two
independent 128-deep products: hi@hi terms pair adjacent contraction strips
(k, k+1); the two cross terms for one strip share one instruction via
stationary slot order (lo,hi) against moving slot order (hi,lo).

Weights are pre-scaled by 2^8 on the host so their hi parts sit in e4m3's
normal range; the scale is folded back in on the Act engine (silu input
scale and the final psum->bf16 copy).  All hi/lo splitting and layout
packing for x/w1/w3/w2 happens on the host in numpy; the device sees fp8
operands laid out partition-major and streams:
  phase A: per h-strip j, per 512-token tile: g,u psums -> silu (ACT),
           h = g*u (DVE) -> h_hi, h_lo fp8 tiles (DVE)
  phase B: per d-strip i: y psum over 12 padded h-strips -> bf16 out.
"""

import sys

sys.path.insert(0, "/opt/trn_rl_repo")

import numpy as np
import ml_dtypes

import concourse.bass as bass
import concourse.mybir as mybir
import concourse.tile as tile
from concourse import bacc
from concourse.bass import ts
from concourse.bass_utils import run_bass_kernel_spmd

F32 = mybir.dt.float32
BF16 = mybir.dt.bfloat16
F8 = mybir.dt.float8e4
NP_F8 = ml_dtypes.float8_e4m3fn
DR = mybir.MatmulPerfMode.DoubleRow
MULT = mybir.AluOpType.mult
SUBTRACT = mybir.AluOpType.subtract

E, H, D, T = 8, 1408, 2048, 16384
TE = T // E            # tokens per expert (uniform fast path)
KD = D // 128          # 16 contraction strips over d
JH = H // 128          # 11 h strips
JH2 = JH + 1           # padded to even for DoubleRow hi@hi pairing in mm2
ID = D // 128          # 16 output d strips
NT = TE // 512         # 4 token tiles of 512
WS = 256.0             # weight pre-scale (2^8) for e4m3 range health
WARMUP_N = 110         # dummy PE matmuls to cover startup DMA + pstate ramp
# spend error budget: drop the hi/lo cross correction on the last N
# contraction strips of the g / u matmuls (4.7us saved per strip; the
# hi@hi term still covers every strip)
DROP_G = 2
DROP_U = 2


def _build_program():
    nc = bacc.Bacc("TRN2", target_bir_lowering=False, debug=False, num_devices=E)

    # [p, k, sl, t]: sl 0=hi, 1=lo of x[t, 128k+p]
    x_d = nc.dram_tensor("xq", [128, KD, 2, TE], F8, kind="ExternalInput").ap()
    # [p, j, s, k, l, m]: s 0=w1,1=w3; l 0=lo,1=hi of (WS*w)[128j+m, 128k+p]
    w13_d = nc.dram_tensor("w13q", [128, JH, 2, KD, 2, 128], F8,
                           kind="ExternalInput").ap()
    # [p, i, kk, l, m]: l 0=lo,1=hi of (WS*w2)[128i+m, 128kk+p]; kk=11 zero pad
    w2_d = nc.dram_tensor("w2q", [128, ID, JH2, 2, 128], F8,
                          kind="ExternalInput").ap()
    # [i, p, t] = y[t, 128i+p]
    y_d = nc.dram_tensor("y", [ID, 128, TE], BF16, kind="ExternalOutput").ap()

    with tile.TileContext(nc) as tc:
        with (
            tc.tile_pool(name="xp", bufs=1) as xp,
            tc.tile_pool(name="wp", bufs=5) as wp,
            tc.tile_pool(name="w2p", bufs=5) as w2p,
            tc.tile_pool(name="hp", bufs=1) as hp,
            tc.tile_pool(name="sp", bufs=2) as sp,
            tc.tile_pool(name="fp", bufs=2) as fp,
            tc.tile_pool(name="yp", bufs=2) as yp,
            tc.tile_pool(name="psA", bufs=2, space="PSUM") as psA,
            tc.tile_pool(name="psB", bufs=3, space="PSUM") as psB,
            tc.tile_pool(name="psW", bufs=1, space="PSUM") as psW,
        ):
            # h strips in fp8 hi/lo; strip JH (=11) is the zero pad for mm2
            # hi@hi pairing (its w2 slot is also zero, but the memset below
            # keeps any stale NaN encodings out of the pair).
            h = hp.tile([128, JH2, 2, TE], F8, tag="h")

            # PE warmup: dummy DoubleRow matmuls on a zeroed fp8 tile keep the
            # PE busy (and its pstate ramping toward full clock) while the
            # first weight/activation DMAs stream in.
            wu = sp.tile([128, 2, 128], F8, tag="wu", bufs=1)
            nc.vector.memset(wu[:], 0.0)
            pw = psW.tile([128, 128], F32, tag="pw")
            for _ in range(WARMUP_N):
                nc.tensor.matmul(pw[:], wu[:], wu[:], start=True, stop=True,
                                 perf_mode=DR)

            # startup DMAs, finest-needed-first so the first real matmuls can
            # begin as soon as possible: w1 strip 0 (g weights), x hi parts of
            # token tile 0, then w3 strip 0 (u weights), x lo, next strips.
            xt = xp.tile([128, KD, 2, TE], F8, tag="xt")
            w13_q = []

            def w13_fetch(j, split=False):
                t = wp.tile([128, 2, KD, 2, 128], F8, tag="w13", name="w13t")
                if split:
                    nc.sync.dma_start(t[:, 0], w13_d[:, j, 0])
                    nc.sync.dma_start(
                        xt[:, :, 0, ts(0, 512)], x_d[:, :, 0, ts(0, 512)]
                    )
                    nc.sync.dma_start(t[:, 1], w13_d[:, j, 1])
                    # x lo parts in two k-halves so the first cross matmuls
                    # can start while the second half streams
                    nc.sync.dma_start(
                        xt[:, :KD // 2, 1, ts(0, 512)],
                        x_d[:, :KD // 2, 1, ts(0, 512)],
                    )
                    nc.sync.dma_start(
                        xt[:, KD // 2:, 1, ts(0, 512)],
                        x_d[:, KD // 2:, 1, ts(0, 512)],
                    )
                else:
                    nc.sync.dma_start(t[:], w13_d[:, j])
                w13_q.append(t)

            w13_fetch(0, split=True)
            for j in (1, 2, 3):
                w13_fetch(j)
            # zero pad strip for mm2 (Pool engine; off the DMA path)
            nc.vector.memset(h[:, JH, 0, :], 0.0)

            def emit_block(w13, j, tt, hh_first):
                tsl = ts(tt, 512)
                pg = psA.tile([128, 512], F32, tag="pg")
                pu = psA.tile([128, 512], F32, tag="pu")
                # hh: hi@hi over strip pairs (k, k+1)
                # cross: stationary (lo,hi) x moving (hi,lo), split in
                # k-halves when hh_first so j0 tracks the split x-lo DMAs
                klast = {0: KD - 1 - DROP_G, 1: KD - 1 - DROP_U}
                if hh_first:
                    phases = [("hh", range(0, KD, 2)),
                              ("cross", range(0, KD // 2)),
                              ("cross", range(KD // 2, KD))]
                else:
                    phases = [("hh", range(0, KD, 2)), ("cross", range(KD))]
                for phase, krange in phases:
                    for s, ps_ in ((0, pg), (1, pu)):
                        for k in krange:
                            if phase == "hh":
                                nc.tensor.matmul(
                                    ps_[:], w13[:, s, k:k + 2, 1, :],
                                    xt[:, k:k + 2, 0, tsl],
                                    start=(k == 0), stop=False, perf_mode=DR,
                                )
                            elif k <= klast[s]:
                                nc.tensor.matmul(
                                    ps_[:], w13[:, s, k, :, :],
                                    xt[:, k, :, tsl],
                                    start=False, stop=(k == klast[s]),
                                    perf_mode=DR,
                                )
                sg = sp.tile([128, 512], F32, tag="sg")
                nc.scalar.activation(
                    sg[:], pg[:], mybir.ActivationFunctionType.Silu,
                    scale=1.0 / WS,
                )
                hf = fp.tile([128, 512], F32, tag="hf")
                # hf = (pu * 1/WS) * sg = u * g
                nc.vector.scalar_tensor_tensor(
                    hf[:], pu[:], 1.0 / WS, sg[:], op0=MULT, op1=MULT
                )
                nc.vector.tensor_copy(h[:, j, 0, tsl], hf[:])
                nc.vector.tensor_sub(h[:, j, 1, tsl], hf[:], h[:, j, 0, tsl])

            # ---- phase A sweep 1: token tile 0 across all strips ----
            # x arrives while the w13 stream feeds the PE; later token tiles
            # stream in between weight strips.
            for j in range(JH):
                if j + 4 < JH:
                    w13_fetch(j + 4)
                emit_block(w13_q[j], j, 0, hh_first=True)
                if j == 0:
                    nc.sync.dma_start(
                        xt[:, :, :, ts(1, 512)], x_d[:, :, :, ts(1, 512)]
                    )
                elif j == 3:
                    nc.sync.dma_start(
                        xt[:, :, :, ts(2, 512)], x_d[:, :, :, ts(2, 512)]
                    )
                elif j == 6:
                    nc.sync.dma_start(
                        xt[:, :, :, ts(3, 512)], x_d[:, :, :, ts(3, 512)]
                    )

            # ---- phase A sweep 2: token tiles 1..3, w13 re-streamed ----
            w13_q.clear()
            w13_fetch(0)
            w13_fetch(1)
            # w2 prefetch (needed only in phase B)
            w2_tiles = []
            for i in range(4):
                w2t = w2p.tile([128, JH2, 2, 128], F8, tag="w2", name="w2t")
                nc.sync.dma_start(w2t[:], w2_d[:, i])
                w2_tiles.append(w2t)
            for j in range(JH):
                if j + 2 < JH:
                    w13_fetch(j + 2)
                for tt in range(1, NT):
                    emit_block(w13_q[j], j, tt, hh_first=False)

            # ---- phase B: y matmuls + store ----
            for i in range(ID):
                if i + 4 < ID:
                    w2t = w2p.tile([128, JH2, 2, 128], F8, tag="w2", name="w2t")
                    nc.sync.dma_start(w2t[:], w2_d[:, i + 4])
                    w2_tiles.append(w2t)
                w2 = w2_tiles[i]
                y_sb = yp.tile([128, TE], BF16, tag="ysb")
                last_i = i == ID - 1
                # the final token tile of the final strip is processed as two
                # 256-token psum groups so its copy/store overlap the
                # preceding matmuls, shrinking the post-last-matmul tail
                chunks = [(tt * 512, 512) for tt in range(NT)]
                if last_i:
                    chunks = chunks[:-1] + [((NT - 1) * 512, 256),
                                            ((NT - 1) * 512 + 256, 256)]
                for ci, (t0, tw) in enumerate(chunks):
                    tsl = ts(t0 // tw, tw)
                    py = psB.tile([128, 512], F32, tag="py")
                    for kk in range(0, JH2, 2):
                        nc.tensor.matmul(
                            py[:, :tw], w2[:, kk:kk + 2, 1, :],
                            h[:, kk:kk + 2, 0, tsl],
                            start=(kk == 0), stop=False, perf_mode=DR,
                        )
                    for kk in range(JH):
                        nc.tensor.matmul(
                            py[:, :tw], w2[:, kk, :, :], h[:, kk, :, tsl],
                            start=False, stop=(kk == JH - 1), perf_mode=DR,
                        )
                    nc.scalar.activation(
                        y_sb[:, tsl], py[:, :tw],
                        mybir.ActivationFunctionType.Copy, scale=1.0 / WS,
                    )
                    if last_i and ci == len(chunks) - 3:
                        # flush all but the final two 256-token chunks early
                        nc.sync.dma_start(
                            y_d[i, :, ts(0, 512 * (NT - 1))],
                            y_sb[:, ts(0, 512 * (NT - 1))],
                        )
                    elif last_i and ci >= len(chunks) - 2:
                        nc.sync.dma_start(y_d[i, :, tsl], y_sb[:, tsl])
                if not last_i:
                    nc.sync.dma_start(y_d[i], y_sb[:])

    nc.compile()
    return nc


_NC = None


def _get_nc():
    global _NC
    if _NC is None:
        _NC = _build_program()
    return _NC


def _hilo(a):
    hi = a.astype(NP_F8)
    lo = (a - hi.astype(np.float32)).astype(NP_F8)
    return hi, lo


def _prep_core_inputs(x_e, w1_e, w3_e, w2_e):
    # xq[p, k, sl, t] with sl=(hi,lo) of x_e[t, 128k+p]
    xr = np.ascontiguousarray(x_e.T).reshape(KD, 128, TE)   # [k, p, t]
    x_hi, x_lo = _hilo(xr)
    xq = np.stack([x_hi, x_lo], axis=2)                      # [k, p, sl, t]
    xq = np.ascontiguousarray(xq.transpose(1, 0, 2, 3))      # [p, k, sl, t]

    # w13q[p, j, s, k, l, m] with l=(lo,hi) of WS*w[128j+m, 128k+p]
    def pack_w13(w):
        r = (w * WS).reshape(JH, 128, KD, 128)               # [j, m, k, p]
        hi, lo = _hilo(r)
        t = np.stack([lo, hi], axis=3)                       # [j, m, k, l, p]
        return t.transpose(4, 0, 2, 3, 1)                    # [p, j, k, l, m]

    w13q = np.stack([pack_w13(w1_e), pack_w13(w3_e)], axis=2)  # [p,j,s,k,l,m]
    w13q = np.ascontiguousarray(w13q)

    # w2q[p, i, kk, l, m] with l=(lo,hi) of WS*w2[128i+m, 128kk+p]; pad kk=11
    r2 = (w2_e * WS).reshape(ID, 128, JH, 128)               # [i, m, kk, p]
    hi2, lo2 = _hilo(r2)
    t2 = np.stack([lo2, hi2], axis=3)                        # [i, m, kk, l, p]
    t2 = t2.transpose(4, 0, 2, 3, 1)                         # [p, i, kk, l, m]
    w2q = np.zeros((128, ID, JH2, 2, 128), dtype=NP_F8)
    w2q[:, :, :JH] = t2
    return {"xq": xq, "w13q": w13q, "w2q": np.ascontiguousarray(w2q)}


def _reference_fallback(w1, w2, w3, x, counts):
    # Exact numpy mirror of the jax reference (incl. scatter-drop / gather-clamp)
    e, h, d = w1.shape
    t = x.shape[0]
    cap = 2 * (t // e)
    counts = counts.astype(np.int64)
    offsets = np.concatenate([[0], np.cumsum(counts)[:-1]])
    eid = np.repeat(np.arange(e), counts)[:t]
    pos = np.arange(t) - offsets[eid]
    buf = np.zeros((e, cap, d), np.float32)
    ok = pos < cap
    buf[eid[ok], pos[ok]] = x[ok]
    out = np.empty((e, cap, d), np.float32)
    for ee in range(e):
        a = buf[ee] @ w1[ee].T
        g = a / (1.0 + np.exp(-a))
        u = buf[ee] @ w3[ee].T
        out[ee] = (g * u) @ w2[ee].T
    pos_c = np.minimum(pos, cap - 1)
    return out[eid, pos_c]


def kernel(w1, w2, w3, x, num_tokens_per_expert):
    w1 = np.asarray(w1, dtype=np.float32)
    w2 = np.asarray(w2, dtype=np.float32)
    w3 = np.asarray(w3, dtype=np.float32)
    x = np.asarray(x, dtype=np.float32)
    counts = np.asarray(num_tokens_per_expert).astype(np.int32)

    if not (x.shape == (T, D) and w1.shape == (E, H, D)
            and np.all(counts == TE)):
        return _reference_fallback(w1, w2, w3, x, counts)

    nc = _get_nc()
    in_maps = []
    for e in range(E):
        in_maps.append(
            _prep_core_inputs(x[e * TE:(e + 1) * TE], w1[e], w3[e], w2[e])
        )
    res = run_bass_kernel_spmd(nc, in_maps, list(range(E)))

    out = np.empty((T, D), dtype=np.float32)
    for e in range(E):
        y = res.results[e]["y"]  # [ID, 128, TE] bf16
        out[e * TE:(e + 1) * TE] = (
            y.astype(np.float32).transpose(2, 0, 1).reshape(TE, D)
        )
    return out
